# revision 39
# baseline (speedup 1.0000x reference)
"""Trainium2 Bass kernel for nn_Attention_59347858096503.

Reference computation (per batch b, head H):
    k = split_heads(key_in @ Wk + bk); q = ...; v = ...
    qsum = q.sum(axis=positions)                      # (b,H,D)
    scores[s] = k[s] . qsum                           # per-key score (no q dep!)
    attn[q,k] = softmax over keys k<=q of scores[k]   # prefix softmax
    ctx[q] = sum_k attn[q,k] v[k];  out = ctx @ Wo + bo

Because scores depend only on the key position, attention reduces to a
prefix-softmax-weighted running average of v:
    ctx[q] = N[q]/Z[q],  N[q] = sum_{k<=q} e(k,q) v[k],  Z[q] = sum e(k,q),
    e(k,q) = exp(scores[k] - m[q]),  m[q] = prefix-max of scores.
Computed blockwise (128 keys/queries per block) with running (M, N, Z) carry.

Sharding: batch 4-way x head-half 2-way = 8 cores. Each core computes its
half of ctx and a partial out = ctx_half @ Wo_half (returned transposed);
host sums the two partials per batch.

Softmax-invariant simplifications: bk drops entirely (constant per-head score
shift); bq enters via qsum; bv and bo are added on host (attn rows sum to 1).
"""
import os
import sys

sys.path.insert(0, "/opt/trn_rl_repo")

import numpy as np
import concourse.bass as bass
import concourse.tile as tile
from concourse import bacc, mybir
from concourse.bass_utils import run_bass_kernel_spmd

F32 = mybir.dt.float32
F32R = mybir.dt.float32r
AF = mybir.ActivationFunctionType
ALU = mybir.AluOpType

S = 1024
D = 1024
NH = 8        # heads per core
HD = 64       # head dim
NB = 8        # key/query blocks of 128
NC = 8        # cores
BIG = 30000.0

CFG = {
    "r_vproj": True,   # fp32r for V projection
    "r_oproj": True,   # fp32r for output projection
    "r_mbcast": True,   # fp32r for m-broadcast (scale cancels in N/Z ratio)
}


def _r(ap, flag):
    return ap.bitcast(F32R) if flag else ap


def build(cfg=CFG, num_devices=NC):
    nc = bacc.Bacc(None, target_bir_lowering=False, debug=False,
                   num_devices=num_devices)

    xqT_d = nc.dram_tensor("xqT", [D, S], F32, kind="ExternalInput")
    xkT_d = nc.dram_tensor("xkT", [D, S], F32, kind="ExternalInput")
    xvT_d = nc.dram_tensor("xvT", [D, S], F32, kind="ExternalInput")
    wq_d = nc.dram_tensor("wq", [D, 512], F32, kind="ExternalInput")
    wkT_d = nc.dram_tensor("wkT", [512, D], F32, kind="ExternalInput")
    wv_d = nc.dram_tensor("wv", [D, 512], F32, kind="ExternalInput")
    wo_d = nc.dram_tensor("wo", [512, D], F32, kind="ExternalInput")
    qb_d = nc.dram_tensor("qbS", [1, 512], F32, kind="ExternalInput")
    mask_d = nc.dram_tensor("masktri", [128, 128], F32, kind="ExternalInput")
    id_d = nc.dram_tensor("ident", [128, 128], F32, kind="ExternalInput")
    bd_d = nc.dram_tensor("bdmask", [128, 4, 8], F32, kind="ExternalInput")
    outT_d = nc.dram_tensor("outT", [D, S], F32, kind="ExternalOutput")

    DTV = F32R if cfg["r_vproj"] else F32
    DTO = F32R if cfg["r_oproj"] else F32

    with tile.TileContext(nc) as tc:
        with (
            tc.tile_pool(name="const", bufs=1) as cpool,
            tc.tile_pool(name="wts", bufs=1) as wpool,
            tc.tile_pool(name="big", bufs=1) as bpool,
            tc.tile_pool(name="sc", bufs=1) as scpool,
            tc.tile_pool(name="xs", bufs=3) as xspool,
            tc.tile_pool(name="gf", bufs=2) as gfpool,
            tc.tile_pool(name="outp", bufs=2) as opool,
            tc.tile_pool(name="ps1", bufs=4, space="PSUM") as ps1,
            tc.tile_pool(name="ps2", bufs=1, space="PSUM") as ps2,
            tc.tile_pool(name="ps3", bufs=2, space="PSUM") as ps3,
        ):
            # ---- constants ----
            ident = cpool.tile([128, 128], F32)
            nc.sync.dma_start(ident[:], id_d.ap())
            masktri = cpool.tile([128, 128], F32)
            nc.sync.dma_start(masktri[:], mask_d.ap())
            bdmask = cpool.tile([128, 4, 8], F32)
            nc.sync.dma_start(bdmask[:], bd_d.ap())
            qb_sb = cpool.tile([1, 512], F32)
            nc.sync.dma_start(qb_sb[:], qb_d.ap())
            ones_row = cpool.tile([1, 128], F32)
            nc.vector.memset(ones_row[:], 1.0)
            if cfg["r_mbcast"]:
                ones_row_r = cpool.tile([1, 128], mybir.dt.bfloat16)
                nc.vector.tensor_copy(ones_row_r[:], ones_row[:])
            else:
                ones_row_r = ones_row
            ones_col = cpool.tile([128, 1], F32)
            nc.vector.memset(ones_col[:], 1.0)

            # ---- score-chain inputs first (critical path) ----
            wq_sb = bpool.tile([128, 8, 512], F32, tag="r1")
            nc.sync.dma_start(wq_sb[:], wq_d.ap().rearrange("(j p) f -> p j f", p=128))
            wkT_sb = wpool.tile([128, 4, 1024], F32)
            nc.sync.dma_start(wkT_sb[:], wkT_d.ap().rearrange("(c p) m -> p c m", p=128))

            # ---- xq stream (ACT ring) -> per-dm-chunk position sums ----
            xqsumT = scpool.tile([128, 8], F32)
            for j in range(8):
                xt = xspool.tile([128, 1024], F32, tag="xq", bufs=2)
                nc.scalar.dma_start(xt[:], xqT_d.ap()[j * 128:(j + 1) * 128, :])
                nc.vector.tensor_reduce(
                    xqsumT[:, j:j + 1], xt[:], mybir.AxisListType.X, ALU.add)

            # ---- V-chain + wo loads: SWDGE cast-DMAs (f32 -> f32r inline),
            # overlapped with the long score-chain front on other queues ----
            xvT_sb = bpool.tile([128, 8, 1024], DTV, tag="r2")
            wv_sb = wpool.tile([128, 8, 512], DTV)
            wo_sb = wpool.tile([128, 4, 1024], DTO)
            nc.gpsimd.dma_start(
                out=wv_sb[:], in_=wv_d.ap().rearrange("(j p) f -> p j f", p=128))
            nc.gpsimd.dma_start(
                out=xvT_sb[:],
                in_=xvT_d.ap().rearrange("(i p) s -> p i s", p=128))
            # ---- qsum (1,512) = xqsum @ wq + S*bq on DVE ----
            qacc = scpool.tile([128, 512], F32)
            nc.vector.memset(qacc[:], 0.0)
            for j in range(8):
                nc.vector.scalar_tensor_tensor(
                    qacc[:], wq_sb[:, j, :], xqsumT[:, j:j + 1], qacc[:],
                    ALU.mult, ALU.add)
            qsum_ps = ps1.tile([128, 512], F32, tag="b1", name="qsum_ps")[0:1, :]
            nc.tensor.matmul(qsum_ps[:], ones_col[:], qacc[:], start=True, stop=True)
            qsum_sb = scpool.tile([1, 512], F32)
            nc.vector.tensor_add(qsum_sb[:], qsum_ps[:], qb_sb[:])

            # ---- qsum row -> columns (128,4) via PE transpose ----
            qt_sb = scpool.tile([128, 4], F32)
            for c in range(4):
                tq = ps1.tile([128, 512], F32, tag="b1")
                nc.tensor.transpose(tq[:, 0:1], qsum_sb[0:1, c * 128:(c + 1) * 128],
                                    ident[0:1, 0:1])
                nc.vector.tensor_copy(qt_sb[:, c:c + 1], tq[:, 0:1])

            # ---- qsumblk[f, h] = qsum[f] * (f//64 == h) ----
            qsblk = scpool.tile([128, 4, 8], F32)
            for c in range(4):
                nc.vector.tensor_scalar_mul(qsblk[:, c, :], bdmask[:, c, :],
                                            qt_sb[:, c:c + 1])

            # ---- u[dm, h] = sum_f wkT[f, dm] qsumblk[f, h] ----
            u_sb = scpool.tile([128, 8, 8], F32)
            for i in range(8):
                u_ps = ps1.tile([128, 512], F32, tag="b1", name="u_ps")[:, 0:8]
                for c in range(4):
                    nc.tensor.matmul(u_ps[:], wkT_sb[:, c, i * 128:(i + 1) * 128],
                                     qsblk[:, c, :], start=(c == 0), stop=(c == 3))
                nc.vector.tensor_copy(u_sb[:, i, :], u_ps[:])

            # ---- scoresT (8, 1024) = u.T @ xkT (xk streamed) ----
            V_sb = bpool.tile([128, 8, 512], F32)
            scoresT_ps = ps2.tile([128, 1024], F32, tag="big2",
                                  name="scoresT_ps")[0:8, :]
            for i in range(8):
                xt = xspool.tile([128, 1024], F32, tag="xk")
                nc.sync.dma_start(xt[:], xkT_d.ap()[i * 128:(i + 1) * 128, :])
                for half in range(2):
                    nc.tensor.matmul(
                        scoresT_ps[:, half * 512:(half + 1) * 512],
                        u_sb[:, i, :], xt[:, half * 512:(half + 1) * 512],
                        start=(i == 0), stop=(i == 7))
            # ---- per-block: scores->sbuf copy, chained cummax, m_flat,
            # scols — so attention block 0 can start before later scores
            # chunks are even processed ----
            scoresT_sb = scpool.tile([8, 1024], F32)
            m_all = scpool.tile([8, 1024], F32)
            m_flat = scpool.tile([1, 8192], mybir.dt.bfloat16 if cfg["r_mbcast"] else F32)
            scols = scpool.tile([128, 64], F32)
            for blk in range(NB):
                sl = slice(blk * 128, (blk + 1) * 128)
                nc.vector.tensor_copy(scoresT_sb[:, sl], scoresT_ps[:, sl])
                init = -3.0e38 if blk == 0 else m_all[:, blk * 128 - 1:blk * 128]
                nc.vector.tensor_tensor_scan(m_all[:, sl], scoresT_sb[:, sl],
                                             scoresT_sb[:, sl], init,
                                             ALU.max, ALU.max)
                nc.gpsimd.dma_start(
                    out=m_flat[0:1, blk * 1024:(blk + 1) * 1024],
                    in_=m_all[:, sl])
                tp = ps1.tile([128, 512], F32, tag="b1")
                nc.tensor.transpose(tp[:, 0:8], scoresT_sb[:, sl],
                                    ident[0:8, 0:8])
                nc.vector.tensor_copy(scols[:, blk * 8:(blk + 1) * 8], tp[:, 0:8])
            # wo load queued after m_flat so it doesn't delay attention start
            nc.gpsimd.dma_start(
                out=wo_sb[:], in_=wo_d.ap().rearrange("(c p) m -> p c m", p=128))

            # ---- attention: blockwise prefix softmax-average of V ----
            ctx_sb = bpool.tile([128, 8, 512], F32, tag="r1")
            NZ = scpool.tile([1, 65 * NH], F32)
            ctxT_sb = bpool.tile([128, 4, 1024], DTO, name="ctxT_sb")

            def _emit_quarter(q):
                n_t = 0
                for c in range(4):
                    for p8 in (2 * q, 2 * q + 1):
                        tp = ps3.tile([128, 512], F32, tag="b2", name="tp")
                        nc.tensor.transpose(
                            tp[:, 0:128], ctx_sb[:, p8, c * 128:(c + 1) * 128],
                            ident[:])
                        dst = ctxT_sb[:, c, p8 * 128:(p8 + 1) * 128]
                        if n_t % 2 == 0:
                            nc.vector.tensor_copy(dst, tp[:, 0:128])
                        else:
                            nc.scalar.copy(dst, tp[:, 0:128])
                        n_t += 1
                for i in range(8):
                    O_ps = ps3.tile([128, 512], F32, tag="b2",
                                    name="O_ps")[:, 0:256]
                    for c in range(4):
                        nc.tensor.matmul(
                            O_ps[:],
                            wo_sb[:, c, i * 128:(i + 1) * 128],
                            ctxT_sb[:, c, q * 256:(q + 1) * 256],
                            start=(c == 0), stop=(c == 3))
                    oT = opool.tile([128, 256], F32, tag="ot")
                    if i % 2 == 0:
                        nc.vector.tensor_copy(oT[:], O_ps[:])
                    else:
                        nc.scalar.copy(oT[:], O_ps[:])
                    nc.sync.dma_start(
                        outT_d.ap()[i * 128:(i + 1) * 128,
                                    q * 256:(q + 1) * 256], oT[:])
            for blk in range(NB):
                G_ps = ps2.tile([128, 1024], F32, tag="big2")
                for half in range(2):
                    nc.tensor.matmul(
                        G_ps[:, half * 512:(half + 1) * 512],
                        ones_row_r[:],
                        m_flat[0:1, blk * 1024 + half * 512:
                               blk * 1024 + (half + 1) * 512],
                        start=True, stop=True)
                # just-in-time V projection for this block (after G so the
                # DVE/ACT pipeline for this block is fed first)
                V_ps = ps1.tile([128, 512], F32, tag="b1", name="V_ps")
                for j in range(8):
                    nc.tensor.matmul(
                        V_ps[:],
                        xvT_sb[:, j, blk * 128:(blk + 1) * 128],
                        wv_sb[:, j, :],
                        start=(j == 0), stop=(j == 7),
                    )
                if blk % 2 == 0:
                    nc.vector.tensor_copy(V_sb[:, blk, :], V_ps[:])
                else:
                    nc.scalar.copy(V_sb[:, blk, :], V_ps[:])
                # G[:, h*128+t] = m_r[t] - s[t'] + mask; one batched exp(-G)
                # per block. m_flat is f32r-rounded and the PE's 1.0*m product
                # is exact, so alpha from m_flat matches the F scale exactly.
                G_sb = gfpool.tile([128, 1024], F32, tag="gsb")
                for h in range(NH):
                    idx = blk * 8 + h
                    nc.vector.scalar_tensor_tensor(
                        G_sb[:, h * 128:(h + 1) * 128],
                        G_ps[:, h * 128:(h + 1) * 128],
                        scols[:, idx:idx + 1], masktri[:],
                        ALU.subtract, ALU.add)
                nc.scalar.activation(G_sb[:], G_sb[:], AF.Exp, scale=-1.0)
                F_all = G_sb
                mb = m_flat
                C_all = ps1.tile([128, 512], F32, tag="b1", name="C_all")
                CZ_all = ps1.tile([128, 512], F32, tag="b1", name="CZ_all")[:, 0:8]
                for h in range(NH):
                    idx = blk * 8 + h
                    base = blk * 1024 + h * 128
                    F_sb = F_all[:, h * 128:(h + 1) * 128]
                    if blk > 0:
                        alpha = gfpool.tile([1, 128], F32, tag="alpha")
                        prev = (blk - 1) * 1024 + h * 128 + 127
                        nc.scalar.activation(alpha[:], m_flat[0:1, base:base + 128],
                                             AF.Exp, scale=-1.0,
                                             bias=m_flat[0:1, prev:prev + 1])
                    # All heads' prefix sums share two PSUM banks (C_all for
                    # the V-parts, CZ_all for the Z columns) so the reciprocal
                    # and the divide batch once per block.
                    nc.tensor.matmul(CZ_all[:, h:h + 1], F_sb, ones_col[:],
                                     start=(h == 0),
                                     stop=(blk == 0 and h == NH - 1))
                    nc.tensor.matmul(C_all[:, h * 64:(h + 1) * 64], F_sb,
                                     V_sb[:, blk, h * 64:(h + 1) * 64],
                                     start=(h == 0),
                                     stop=(blk == 0 and h == NH - 1))
                    # next-block carry row: full-block sums at scale M_new
                    # (not needed after the last block)
                    if blk < NB - 1:
                        NZr_ps = ps1.tile([128, 512], F32, tag="b1",
                                          name="NZr_ps")[0:1, 0:65]
                        Fcol = F_all[:, h * 128 + 127:h * 128 + 128]
                        nc.tensor.matmul(NZr_ps[0:1, 0:64], Fcol,
                                         V_sb[:, blk, h * 64:(h + 1) * 64],
                                         start=True, stop=False)
                        nc.tensor.matmul(NZr_ps[0:1, 64:65], Fcol, ones_col[:],
                                         start=False, stop=(blk == 0))
                    if blk > 0:
                        nc.tensor.matmul(C_all[:, h * 64:(h + 1) * 64], alpha[:],
                                         NZ[0:1, h * 65:h * 65 + 64],
                                         start=False, stop=(h == NH - 1))
                        nc.tensor.matmul(CZ_all[:, h:h + 1], alpha[:],
                                         NZ[0:1, h * 65 + 64:h * 65 + 65],
                                         start=False, stop=(h == NH - 1))
                        if blk < NB - 1:
                            nc.tensor.matmul(NZr_ps[0:1, 0:65], alpha[0:1, 127:128],
                                             NZ[0:1, h * 65:(h + 1) * 65],
                                             start=False, stop=True)
                    if blk < NB - 1:
                        nc.vector.tensor_copy(NZ[0:1, h * 65:(h + 1) * 65], NZr_ps[:])
                zr_all = gfpool.tile([128, 8], F32, tag="zr")
                nc.vector.reciprocal(zr_all[:], CZ_all[:, 0:8])
                for h in range(NH):
                    if h % 2 == 0:
                        nc.scalar.mul(ctx_sb[:, blk, h * 64:(h + 1) * 64],
                                      C_all[:, h * 64:(h + 1) * 64],
                                      zr_all[:, h:h + 1])
                    else:
                        nc.vector.tensor_scalar_mul(
                            ctx_sb[:, blk, h * 64:(h + 1) * 64],
                            C_all[:, h * 64:(h + 1) * 64], zr_all[:, h:h + 1])

                # Out-projection quarters, emitted one block late so their
                # ctx inputs are already finalized (no PE stall) and the PE
                # work overlaps later blocks' DVE/ACT attention work.
                if blk >= 2 and blk % 2 == 0:
                    _emit_quarter(blk // 2 - 1)
            _emit_quarter(3)

    nc.compile()
    return nc


_NC_CACHE = {}


def _get_nc():
    key = tuple(sorted(CFG.items()))
    if key not in _NC_CACHE:
        _NC_CACHE[key] = build(CFG)
    return _NC_CACHE[key]


def _consts():
    p = np.arange(128)
    masktri = np.where(p[:, None] > p[None, :], BIG, 0.0).astype(np.float32)
    ident = np.eye(128, dtype=np.float32)
    bd = np.zeros((128, 4, 8), np.float32)
    for c in range(4):
        for pp in range(128):
            bd[pp, c, 2 * c + pp // 64] = 1.0
    return masktri, ident, bd


def make_in_maps(key_in, query_in, value_in, Wk, bk, Wq, bq, Wv, bv, Wo, bo):
    masktri, ident, bd = _consts()
    maps = []
    for core in range(NC):
        b, hh = core // 2, core % 2
        sl = slice(hh * 512, (hh + 1) * 512)
        maps.append({
            "xqT": np.ascontiguousarray(np.asarray(query_in[b]).T, np.float32),
            "xkT": np.ascontiguousarray(np.asarray(key_in[b]).T, np.float32),
            "xvT": np.ascontiguousarray(np.asarray(value_in[b]).T, np.float32),
            "wq": np.ascontiguousarray(np.asarray(Wq)[:, sl], np.float32),
            "wkT": np.ascontiguousarray(np.asarray(Wk)[:, sl].T, np.float32),
            "wv": np.ascontiguousarray(np.asarray(Wv)[:, sl], np.float32),
            "wo": np.ascontiguousarray(np.asarray(Wo)[sl, :], np.float32),
            "qbS": (S * np.asarray(bq)[sl]).reshape(1, 512).astype(np.float32),
            "masktri": masktri, "ident": ident, "bdmask": bd,
        })
    return maps


def run(inputs, trace=False):
    nc = _get_nc()
    in_maps = make_in_maps(**inputs)
    try:
        res = run_bass_kernel_spmd(nc, in_maps, list(range(NC)), trace=trace)
    except ModuleNotFoundError:
        # Tracing needs antenv.axon_hooks, absent in some containers; retry
        # with tracing suppressed (BASS_TRACE in the env would re-trigger it).
        os.environ["BASS_NEVER_TRACE"] = "1"
        res = run_bass_kernel_spmd(nc, in_maps, list(range(NC)), trace=False)
    Wo = np.asarray(inputs["Wo"], np.float32)
    extra = (np.asarray(inputs["bv"], np.float32) @ Wo
             + np.asarray(inputs["bo"], np.float32)).astype(np.float32)
    out = np.empty((4, S, D), np.float32)
    for b in range(4):
        out[b] = (res.results[2 * b]["outT"].T + res.results[2 * b + 1]["outT"].T
                  + extra)
    return out, res


def kernel(**inputs):
    out, _ = run(inputs, trace=False)
    return out


# revision 40
# speedup vs baseline: 1.0124x; 1.0124x over previous
"""Trainium2 Bass kernel for nn_Attention_59347858096503.

Reference computation (per batch b, head H):
    k = split_heads(key_in @ Wk + bk); q = ...; v = ...
    qsum = q.sum(axis=positions)                      # (b,H,D)
    scores[s] = k[s] . qsum                           # per-key score (no q dep!)
    attn[q,k] = softmax over keys k<=q of scores[k]   # prefix softmax
    ctx[q] = sum_k attn[q,k] v[k];  out = ctx @ Wo + bo

Because scores depend only on the key position, attention reduces to a
prefix-softmax-weighted running average of v:
    ctx[q] = N[q]/Z[q],  N[q] = sum_{k<=q} e(k,q) v[k],  Z[q] = sum e(k,q),
    e(k,q) = exp(scores[k] - m[q]),  m[q] = prefix-max of scores.
Computed blockwise (128 keys/queries per block) with running (M, N, Z) carry.

Sharding: batch 4-way x head-half 2-way = 8 cores. Each core computes its
half of ctx and a partial out = ctx_half @ Wo_half (returned transposed);
host sums the two partials per batch.

Softmax-invariant simplifications: bk drops entirely (constant per-head score
shift); bq enters via qsum; bv and bo are added on host (attn rows sum to 1).
"""
import os
import sys

sys.path.insert(0, "/opt/trn_rl_repo")

import numpy as np
import concourse.bass as bass
import concourse.tile as tile
from concourse import bacc, mybir
from concourse.bass_utils import run_bass_kernel_spmd

F32 = mybir.dt.float32
F32R = mybir.dt.float32r
AF = mybir.ActivationFunctionType
ALU = mybir.AluOpType

S = 1024
D = 1024
NH = 8        # heads per core
HD = 64       # head dim
NB = 8        # key/query blocks of 128
NC = 8        # cores
BIG = 30000.0

CFG = {
    "r_vproj": True,   # fp32r for V projection
    "r_oproj": True,   # fp32r for output projection
    "r_mbcast": True,   # fp32r for m-broadcast (scale cancels in N/Z ratio)
}


def _r(ap, flag):
    return ap.bitcast(F32R) if flag else ap


def build(cfg=CFG, num_devices=NC):
    nc = bacc.Bacc(None, target_bir_lowering=False, debug=False,
                   num_devices=num_devices)

    xqT_d = nc.dram_tensor("xqT", [D, S], F32, kind="ExternalInput")
    xkT_d = nc.dram_tensor("xkT", [D, S], F32, kind="ExternalInput")
    xvT_d = nc.dram_tensor("xvT", [D, S], F32, kind="ExternalInput")
    wq_d = nc.dram_tensor("wq", [D, 512], F32, kind="ExternalInput")
    wkT_d = nc.dram_tensor("wkT", [512, D], F32, kind="ExternalInput")
    wv_d = nc.dram_tensor("wv", [D, 512], F32, kind="ExternalInput")
    wo_d = nc.dram_tensor("wo", [512, D], F32, kind="ExternalInput")
    qb_d = nc.dram_tensor("qbS", [1, 512], F32, kind="ExternalInput")
    mask_d = nc.dram_tensor("masktri", [128, 128], F32, kind="ExternalInput")
    id_d = nc.dram_tensor("ident", [128, 128], F32, kind="ExternalInput")
    bd_d = nc.dram_tensor("bdmask", [128, 4, 8], F32, kind="ExternalInput")
    outT_d = nc.dram_tensor("outT", [D, S], F32, kind="ExternalOutput")

    DTV = F32R if cfg["r_vproj"] else F32
    DTO = F32R if cfg["r_oproj"] else F32

    with tile.TileContext(nc) as tc:
        with (
            tc.tile_pool(name="const", bufs=1) as cpool,
            tc.tile_pool(name="wts", bufs=1) as wpool,
            tc.tile_pool(name="big", bufs=1) as bpool,
            tc.tile_pool(name="sc", bufs=1) as scpool,
            tc.tile_pool(name="xs", bufs=3) as xspool,
            tc.tile_pool(name="gf", bufs=2) as gfpool,
            tc.tile_pool(name="outp", bufs=2) as opool,
            tc.tile_pool(name="ps1", bufs=4, space="PSUM") as ps1,
            tc.tile_pool(name="ps2", bufs=2, space="PSUM") as ps2,
            tc.tile_pool(name="ps3", bufs=2, space="PSUM") as ps3,
        ):
            # ---- constants ----
            ident = cpool.tile([128, 128], F32)
            nc.sync.dma_start(ident[:], id_d.ap())
            masktri = cpool.tile([128, 128], F32)
            nc.sync.dma_start(masktri[:], mask_d.ap())
            bdmask = cpool.tile([128, 4, 8], F32)
            nc.sync.dma_start(bdmask[:], bd_d.ap())
            qb_sb = cpool.tile([1, 512], F32)
            nc.sync.dma_start(qb_sb[:], qb_d.ap())
            ones_row = cpool.tile([1, 128], F32)
            nc.vector.memset(ones_row[:], 1.0)
            if cfg["r_mbcast"]:
                ones_row_r = cpool.tile([1, 128], mybir.dt.bfloat16)
                nc.vector.tensor_copy(ones_row_r[:], ones_row[:])
            else:
                ones_row_r = ones_row
            ones_col = cpool.tile([128, 1], F32)
            nc.vector.memset(ones_col[:], 1.0)

            # ---- score-chain inputs first (critical path) ----
            wq_sb = bpool.tile([128, 8, 512], F32, tag="r1")
            nc.sync.dma_start(wq_sb[:], wq_d.ap().rearrange("(j p) f -> p j f", p=128))
            wkT_sb = wpool.tile([128, 4, 1024], F32)
            nc.sync.dma_start(wkT_sb[:], wkT_d.ap().rearrange("(c p) m -> p c m", p=128))

            # ---- xq stream (ACT ring) -> per-dm-chunk position sums ----
            xqsumT = scpool.tile([128, 8], F32)
            for j in range(8):
                xt = xspool.tile([128, 1024], F32, tag="xq", bufs=2)
                nc.scalar.dma_start(xt[:], xqT_d.ap()[j * 128:(j + 1) * 128, :])
                nc.vector.tensor_reduce(
                    xqsumT[:, j:j + 1], xt[:], mybir.AxisListType.X, ALU.add)

            # ---- V-chain + wo loads: SWDGE cast-DMAs (f32 -> f32r inline),
            # overlapped with the long score-chain front on other queues ----
            xvT_sb = bpool.tile([128, 8, 1024], DTV, tag="r2")
            wv_sb = wpool.tile([128, 8, 512], DTV)
            wo_sb = wpool.tile([128, 4, 1024], DTO)
            nc.gpsimd.dma_start(
                out=wv_sb[:], in_=wv_d.ap().rearrange("(j p) f -> p j f", p=128))
            nc.gpsimd.dma_start(
                out=xvT_sb[:],
                in_=xvT_d.ap().rearrange("(i p) s -> p i s", p=128))
            # ---- qsum (1,512) = xqsum @ wq + S*bq on DVE ----
            qacc = scpool.tile([128, 512], F32)
            nc.vector.memset(qacc[:], 0.0)
            for j in range(8):
                nc.vector.scalar_tensor_tensor(
                    qacc[:], wq_sb[:, j, :], xqsumT[:, j:j + 1], qacc[:],
                    ALU.mult, ALU.add)
            qsum_ps = ps1.tile([128, 512], F32, tag="b1", name="qsum_ps")[0:1, :]
            nc.tensor.matmul(qsum_ps[:], ones_col[:], qacc[:], start=True, stop=True)
            qsum_sb = scpool.tile([1, 512], F32)
            nc.vector.tensor_add(qsum_sb[:], qsum_ps[:], qb_sb[:])

            # ---- qsum row -> columns (128,4) via PE transpose ----
            qt_sb = scpool.tile([128, 4], F32)
            for c in range(4):
                tq = ps1.tile([128, 512], F32, tag="b1")
                nc.tensor.transpose(tq[:, 0:1], qsum_sb[0:1, c * 128:(c + 1) * 128],
                                    ident[0:1, 0:1])
                nc.vector.tensor_copy(qt_sb[:, c:c + 1], tq[:, 0:1])

            # ---- qsumblk[f, h] = qsum[f] * (f//64 == h) ----
            qsblk = scpool.tile([128, 4, 8], F32)
            for c in range(4):
                nc.vector.tensor_scalar_mul(qsblk[:, c, :], bdmask[:, c, :],
                                            qt_sb[:, c:c + 1])

            # ---- u[dm, h] = sum_f wkT[f, dm] qsumblk[f, h] ----
            u_sb = scpool.tile([128, 8, 8], F32)
            for i in range(8):
                u_ps = ps1.tile([128, 512], F32, tag="b1", name="u_ps")[:, 0:8]
                for c in range(4):
                    nc.tensor.matmul(u_ps[:], wkT_sb[:, c, i * 128:(i + 1) * 128],
                                     qsblk[:, c, :], start=(c == 0), stop=(c == 3))
                nc.vector.tensor_copy(u_sb[:, i, :], u_ps[:])

            # ---- scoresT (8, 1024) = u.T @ xkT (xk streamed) ----
            V_sb = bpool.tile([128, 8, 512], F32)
            scoresT_psA = ps2.tile([128, 512], F32, tag="big2",
                                   name="scoresT_psA")[0:8, :]
            scoresT_psB = ps2.tile([128, 512], F32, tag="big2",
                                   name="scoresT_psB")[0:8, :]
            sc_half = (scoresT_psA, scoresT_psB)
            for i in range(8):
                xt = xspool.tile([128, 1024], F32, tag="xk")
                nc.sync.dma_start(xt[:], xkT_d.ap()[i * 128:(i + 1) * 128, :])
                for half in range(2):
                    nc.tensor.matmul(
                        sc_half[half][:],
                        u_sb[:, i, :], xt[:, half * 512:(half + 1) * 512],
                        start=(i == 0), stop=(i == 7))
            # ---- per-block: scores->sbuf copy, chained cummax, m_flat,
            # scols — so attention block 0 can start before later scores
            # chunks are even processed ----
            scoresT_sb = scpool.tile([8, 1024], F32)
            m_all = scpool.tile([8, 1024], F32)
            m_flat = scpool.tile([1, 8192], mybir.dt.bfloat16 if cfg["r_mbcast"] else F32)
            scols = scpool.tile([128, 64], F32)
            for blk in range(NB):
                sl = slice(blk * 128, (blk + 1) * 128)
                hsl = slice((blk % 4) * 128, (blk % 4 + 1) * 128)
                nc.vector.tensor_copy(scoresT_sb[:, sl], sc_half[blk // 4][:, hsl])
                init = -3.0e38 if blk == 0 else m_all[:, blk * 128 - 1:blk * 128]
                nc.vector.tensor_tensor_scan(m_all[:, sl], scoresT_sb[:, sl],
                                             scoresT_sb[:, sl], init,
                                             ALU.max, ALU.max)
                nc.gpsimd.dma_start(
                    out=m_flat[0:1, blk * 1024:(blk + 1) * 1024],
                    in_=m_all[:, sl])
                tp = ps1.tile([128, 512], F32, tag="b1")
                nc.tensor.transpose(tp[:, 0:8], scoresT_sb[:, sl],
                                    ident[0:8, 0:8])
                nc.vector.tensor_copy(scols[:, blk * 8:(blk + 1) * 8], tp[:, 0:8])
            # wo load queued after m_flat so it doesn't delay attention start
            nc.gpsimd.dma_start(
                out=wo_sb[:], in_=wo_d.ap().rearrange("(c p) m -> p c m", p=128))

            # ---- attention: blockwise prefix softmax-average of V ----
            ctx_sb = bpool.tile([128, 8, 512], F32, tag="r1")
            NZ = scpool.tile([1, 65 * NH], F32)
            ctxT_sb = bpool.tile([128, 4, 1024], DTO, name="ctxT_sb")

            def _emit_quarter(q):
                n_t = 0
                for c in range(4):
                    for p8 in (2 * q, 2 * q + 1):
                        tp = ps3.tile([128, 512], F32, tag="b2", name="tp")
                        nc.tensor.transpose(
                            tp[:, 0:128], ctx_sb[:, p8, c * 128:(c + 1) * 128],
                            ident[:])
                        dst = ctxT_sb[:, c, p8 * 128:(p8 + 1) * 128]
                        if n_t % 2 == 0:
                            nc.vector.tensor_copy(dst, tp[:, 0:128])
                        else:
                            nc.scalar.copy(dst, tp[:, 0:128])
                        n_t += 1
                for i in range(8):
                    O_ps = ps3.tile([128, 512], F32, tag="b2",
                                    name="O_ps")[:, 0:256]
                    for c in range(4):
                        nc.tensor.matmul(
                            O_ps[:],
                            wo_sb[:, c, i * 128:(i + 1) * 128],
                            ctxT_sb[:, c, q * 256:(q + 1) * 256],
                            start=(c == 0), stop=(c == 3))
                    oT = opool.tile([128, 256], F32, tag="ot")
                    if i % 2 == 0:
                        nc.vector.tensor_copy(oT[:], O_ps[:])
                    else:
                        nc.scalar.copy(oT[:], O_ps[:])
                    nc.sync.dma_start(
                        outT_d.ap()[i * 128:(i + 1) * 128,
                                    q * 256:(q + 1) * 256], oT[:])
            for blk in range(NB):
                G_h = []
                for half in range(2):
                    G_ps = ps2.tile([128, 512], F32, tag="big2", name="G_ps")
                    nc.tensor.matmul(
                        G_ps[:],
                        ones_row_r[:],
                        m_flat[0:1, blk * 1024 + half * 512:
                               blk * 1024 + (half + 1) * 512],
                        start=True, stop=True)
                    G_h.append(G_ps)
                # just-in-time V projection for this block (after G so the
                # DVE/ACT pipeline for this block is fed first)
                V_ps = ps1.tile([128, 512], F32, tag="b1", name="V_ps")
                for j in range(8):
                    nc.tensor.matmul(
                        V_ps[:],
                        xvT_sb[:, j, blk * 128:(blk + 1) * 128],
                        wv_sb[:, j, :],
                        start=(j == 0), stop=(j == 7),
                    )
                if blk % 2 == 0:
                    nc.vector.tensor_copy(V_sb[:, blk, :], V_ps[:])
                else:
                    nc.scalar.copy(V_sb[:, blk, :], V_ps[:])
                # G[:, h*128+t] = m_r[t] - s[t'] + mask; one batched exp(-G)
                # per block. m_flat is f32r-rounded and the PE's 1.0*m product
                # is exact, so alpha from m_flat matches the F scale exactly.
                G_sb = gfpool.tile([128, 1024], F32, tag="gsb")
                for h in range(NH):
                    idx = blk * 8 + h
                    nc.vector.scalar_tensor_tensor(
                        G_sb[:, h * 128:(h + 1) * 128],
                        G_h[h // 4][:, (h % 4) * 128:(h % 4 + 1) * 128],
                        scols[:, idx:idx + 1], masktri[:],
                        ALU.subtract, ALU.add)
                nc.scalar.activation(G_sb[:], G_sb[:], AF.Exp, scale=-1.0)
                F_all = G_sb
                mb = m_flat
                C_all = ps1.tile([128, 512], F32, tag="b1", name="C_all")
                CZ_all = ps1.tile([128, 512], F32, tag="b1", name="CZ_all")[:, 0:8]
                for h in range(NH):
                    idx = blk * 8 + h
                    base = blk * 1024 + h * 128
                    F_sb = F_all[:, h * 128:(h + 1) * 128]
                    if blk > 0:
                        alpha = gfpool.tile([1, 128], F32, tag="alpha")
                        prev = (blk - 1) * 1024 + h * 128 + 127
                        nc.scalar.activation(alpha[:], m_flat[0:1, base:base + 128],
                                             AF.Exp, scale=-1.0,
                                             bias=m_flat[0:1, prev:prev + 1])
                    # All heads' prefix sums share two PSUM banks (C_all for
                    # the V-parts, CZ_all for the Z columns) so the reciprocal
                    # and the divide batch once per block.
                    nc.tensor.matmul(CZ_all[:, h:h + 1], F_sb, ones_col[:],
                                     start=(h == 0),
                                     stop=(blk == 0 and h == NH - 1))
                    nc.tensor.matmul(C_all[:, h * 64:(h + 1) * 64], F_sb,
                                     V_sb[:, blk, h * 64:(h + 1) * 64],
                                     start=(h == 0),
                                     stop=(blk == 0 and h == NH - 1))
                    # next-block carry row: full-block sums at scale M_new
                    # (not needed after the last block)
                    if blk < NB - 1:
                        NZr_ps = ps1.tile([128, 512], F32, tag="b1",
                                          name="NZr_ps")[0:1, 0:65]
                        Fcol = F_all[:, h * 128 + 127:h * 128 + 128]
                        nc.tensor.matmul(NZr_ps[0:1, 0:64], Fcol,
                                         V_sb[:, blk, h * 64:(h + 1) * 64],
                                         start=True, stop=False)
                        nc.tensor.matmul(NZr_ps[0:1, 64:65], Fcol, ones_col[:],
                                         start=False, stop=(blk == 0))
                    if blk > 0:
                        nc.tensor.matmul(C_all[:, h * 64:(h + 1) * 64], alpha[:],
                                         NZ[0:1, h * 65:h * 65 + 64],
                                         start=False, stop=(h == NH - 1))
                        nc.tensor.matmul(CZ_all[:, h:h + 1], alpha[:],
                                         NZ[0:1, h * 65 + 64:h * 65 + 65],
                                         start=False, stop=(h == NH - 1))
                        if blk < NB - 1:
                            nc.tensor.matmul(NZr_ps[0:1, 0:65], alpha[0:1, 127:128],
                                             NZ[0:1, h * 65:(h + 1) * 65],
                                             start=False, stop=True)
                    if blk < NB - 1:
                        nc.vector.tensor_copy(NZ[0:1, h * 65:(h + 1) * 65], NZr_ps[:])
                zr_all = gfpool.tile([128, 8], F32, tag="zr")
                nc.vector.reciprocal(zr_all[:], CZ_all[:, 0:8])
                for h in range(NH):
                    if h % 2 == 0:
                        nc.scalar.mul(ctx_sb[:, blk, h * 64:(h + 1) * 64],
                                      C_all[:, h * 64:(h + 1) * 64],
                                      zr_all[:, h:h + 1])
                    else:
                        nc.vector.tensor_scalar_mul(
                            ctx_sb[:, blk, h * 64:(h + 1) * 64],
                            C_all[:, h * 64:(h + 1) * 64], zr_all[:, h:h + 1])

                # Out-projection quarters, emitted one block late so their
                # ctx inputs are already finalized (no PE stall) and the PE
                # work overlaps later blocks' DVE/ACT attention work.
                if blk >= 2 and blk % 2 == 0:
                    _emit_quarter(blk // 2 - 1)
            _emit_quarter(3)

    nc.compile()
    return nc


_NC_CACHE = {}


def _get_nc():
    key = tuple(sorted(CFG.items()))
    if key not in _NC_CACHE:
        _NC_CACHE[key] = build(CFG)
    return _NC_CACHE[key]


def _consts():
    p = np.arange(128)
    masktri = np.where(p[:, None] > p[None, :], BIG, 0.0).astype(np.float32)
    ident = np.eye(128, dtype=np.float32)
    bd = np.zeros((128, 4, 8), np.float32)
    for c in range(4):
        for pp in range(128):
            bd[pp, c, 2 * c + pp // 64] = 1.0
    return masktri, ident, bd


def make_in_maps(key_in, query_in, value_in, Wk, bk, Wq, bq, Wv, bv, Wo, bo):
    masktri, ident, bd = _consts()
    maps = []
    for core in range(NC):
        b, hh = core // 2, core % 2
        sl = slice(hh * 512, (hh + 1) * 512)
        maps.append({
            "xqT": np.ascontiguousarray(np.asarray(query_in[b]).T, np.float32),
            "xkT": np.ascontiguousarray(np.asarray(key_in[b]).T, np.float32),
            "xvT": np.ascontiguousarray(np.asarray(value_in[b]).T, np.float32),
            "wq": np.ascontiguousarray(np.asarray(Wq)[:, sl], np.float32),
            "wkT": np.ascontiguousarray(np.asarray(Wk)[:, sl].T, np.float32),
            "wv": np.ascontiguousarray(np.asarray(Wv)[:, sl], np.float32),
            "wo": np.ascontiguousarray(np.asarray(Wo)[sl, :], np.float32),
            "qbS": (S * np.asarray(bq)[sl]).reshape(1, 512).astype(np.float32),
            "masktri": masktri, "ident": ident, "bdmask": bd,
        })
    return maps


def run(inputs, trace=False):
    nc = _get_nc()
    in_maps = make_in_maps(**inputs)
    try:
        res = run_bass_kernel_spmd(nc, in_maps, list(range(NC)), trace=trace)
    except ModuleNotFoundError:
        # Tracing needs antenv.axon_hooks, absent in some containers; retry
        # with tracing suppressed (BASS_TRACE in the env would re-trigger it).
        os.environ["BASS_NEVER_TRACE"] = "1"
        res = run_bass_kernel_spmd(nc, in_maps, list(range(NC)), trace=False)
    Wo = np.asarray(inputs["Wo"], np.float32)
    extra = (np.asarray(inputs["bv"], np.float32) @ Wo
             + np.asarray(inputs["bo"], np.float32)).astype(np.float32)
    out = np.empty((4, S, D), np.float32)
    for b in range(4):
        out[b] = (res.results[2 * b]["outT"].T + res.results[2 * b + 1]["outT"].T
                  + extra)
    return out, res


def kernel(**inputs):
    out, _ = run(inputs, trace=False)
    return out


# revision 41
# speedup vs baseline: 1.0233x; 1.0108x over previous
"""Trainium2 Bass kernel for nn_Attention_59347858096503.

Reference computation (per batch b, head H):
    k = split_heads(key_in @ Wk + bk); q = ...; v = ...
    qsum = q.sum(axis=positions)                      # (b,H,D)
    scores[s] = k[s] . qsum                           # per-key score (no q dep!)
    attn[q,k] = softmax over keys k<=q of scores[k]   # prefix softmax
    ctx[q] = sum_k attn[q,k] v[k];  out = ctx @ Wo + bo

Because scores depend only on the key position, attention reduces to a
prefix-softmax-weighted running average of v:
    ctx[q] = N[q]/Z[q],  N[q] = sum_{k<=q} e(k,q) v[k],  Z[q] = sum e(k,q),
    e(k,q) = exp(scores[k] - m[q]),  m[q] = prefix-max of scores.
Computed blockwise (128 keys/queries per block) with running (M, N, Z) carry.

Sharding: batch 4-way x head-half 2-way = 8 cores. Each core computes its
half of ctx and a partial out = ctx_half @ Wo_half (returned transposed);
host sums the two partials per batch.

Softmax-invariant simplifications: bk drops entirely (constant per-head score
shift); bq enters via qsum; bv and bo are added on host (attn rows sum to 1).
"""
import os
import sys

sys.path.insert(0, "/opt/trn_rl_repo")

import numpy as np
import concourse.bass as bass
import concourse.tile as tile
from concourse import bacc, mybir
from concourse.bass_utils import run_bass_kernel_spmd

F32 = mybir.dt.float32
F32R = mybir.dt.float32r
AF = mybir.ActivationFunctionType
ALU = mybir.AluOpType

S = 1024
D = 1024
NH = 8        # heads per core
HD = 64       # head dim
NB = 8        # key/query blocks of 128
NC = 8        # cores
BIG = 30000.0

CFG = {
    "r_vproj": True,   # fp32r for V projection
    "r_oproj": True,   # fp32r for output projection
    "r_mbcast": True,   # fp32r for m-broadcast (scale cancels in N/Z ratio)
}


def _r(ap, flag):
    return ap.bitcast(F32R) if flag else ap


def build(cfg=CFG, num_devices=NC):
    nc = bacc.Bacc(None, target_bir_lowering=False, debug=False,
                   num_devices=num_devices)

    xqT_d = nc.dram_tensor("xqT", [D, S], F32, kind="ExternalInput")
    xkT_d = nc.dram_tensor("xkT", [D, S], F32, kind="ExternalInput")
    xvT_d = nc.dram_tensor("xvT", [D, S], F32, kind="ExternalInput")
    wq_d = nc.dram_tensor("wq", [D, 512], F32, kind="ExternalInput")
    wkT_d = nc.dram_tensor("wkT", [512, D], F32, kind="ExternalInput")
    wv_d = nc.dram_tensor("wv", [D, 512], F32, kind="ExternalInput")
    wo_d = nc.dram_tensor("wo", [512, D], F32, kind="ExternalInput")
    qb_d = nc.dram_tensor("qbS", [1, 512], F32, kind="ExternalInput")
    mask_d = nc.dram_tensor("masktri", [128, 128], F32, kind="ExternalInput")
    id_d = nc.dram_tensor("ident", [128, 128], F32, kind="ExternalInput")
    bd_d = nc.dram_tensor("bdmask", [128, 4, 8], F32, kind="ExternalInput")
    outT_d = nc.dram_tensor("outT", [D, S], F32, kind="ExternalOutput")

    DTV = F32R if cfg["r_vproj"] else F32
    DTO = F32R if cfg["r_oproj"] else F32

    with tile.TileContext(nc) as tc:
        with (
            tc.tile_pool(name="const", bufs=1) as cpool,
            tc.tile_pool(name="wts", bufs=1) as wpool,
            tc.tile_pool(name="big", bufs=1) as bpool,
            tc.tile_pool(name="sc", bufs=1) as scpool,
            tc.tile_pool(name="xs", bufs=3) as xspool,
            tc.tile_pool(name="gf", bufs=2) as gfpool,
            tc.tile_pool(name="outp", bufs=2) as opool,
            tc.tile_pool(name="ps1", bufs=4, space="PSUM") as ps1,
            tc.tile_pool(name="ps2", bufs=2, space="PSUM") as ps2,
            tc.tile_pool(name="ps3", bufs=2, space="PSUM") as ps3,
        ):
            # ---- constants ----
            ident = cpool.tile([128, 128], F32)
            nc.sync.dma_start(ident[:], id_d.ap())
            masktri = cpool.tile([128, 128], F32)
            nc.sync.dma_start(masktri[:], mask_d.ap())
            bdmask = cpool.tile([128, 4, 8], F32)
            nc.sync.dma_start(bdmask[:], bd_d.ap())
            qb_sb = cpool.tile([1, 512], F32)
            nc.sync.dma_start(qb_sb[:], qb_d.ap())
            ones_row = cpool.tile([1, 128], F32)
            nc.vector.memset(ones_row[:], 1.0)
            if cfg["r_mbcast"]:
                ones_row_r = cpool.tile([1, 128], mybir.dt.bfloat16)
                nc.vector.tensor_copy(ones_row_r[:], ones_row[:])
            else:
                ones_row_r = ones_row
            ones_col = cpool.tile([128, 1], F32)
            nc.vector.memset(ones_col[:], 1.0)

            # ---- score-chain inputs first (critical path) ----
            wq_sb = bpool.tile([128, 8, 512], F32, tag="r1")
            nc.sync.dma_start(wq_sb[:], wq_d.ap().rearrange("(j p) f -> p j f", p=128))
            wkT_sb = wpool.tile([128, 4, 1024], F32)
            nc.sync.dma_start(wkT_sb[:], wkT_d.ap().rearrange("(c p) m -> p c m", p=128))

            # ---- xq stream (ACT ring) -> per-dm-chunk position sums ----
            xqsumT = scpool.tile([128, 8], F32)
            for j in range(8):
                xt = xspool.tile([128, 1024], F32, tag="xq", bufs=2)
                nc.scalar.dma_start(xt[:], xqT_d.ap()[j * 128:(j + 1) * 128, :])
                nc.vector.tensor_reduce(
                    xqsumT[:, j:j + 1], xt[:], mybir.AxisListType.X, ALU.add)

            # ---- V-chain + wo loads: SWDGE cast-DMAs (f32 -> f32r inline),
            # overlapped with the long score-chain front on other queues ----
            xvT_sb = bpool.tile([128, 8, 1024], DTV, tag="r2")
            wv_sb = wpool.tile([128, 8, 512], DTV)
            wo_sb = wpool.tile([128, 4, 1024], DTO)
            nc.gpsimd.dma_start(
                out=wv_sb[:], in_=wv_d.ap().rearrange("(j p) f -> p j f", p=128))
            nc.gpsimd.dma_start(
                out=xvT_sb[:],
                in_=xvT_d.ap().rearrange("(i p) s -> p i s", p=128))
            # ---- qsum (1,512) = xqsum @ wq + S*bq on DVE ----
            qacc = scpool.tile([128, 512], F32)
            nc.vector.memset(qacc[:], 0.0)
            for j in range(8):
                nc.vector.scalar_tensor_tensor(
                    qacc[:], wq_sb[:, j, :], xqsumT[:, j:j + 1], qacc[:],
                    ALU.mult, ALU.add)
            qsum_ps = ps1.tile([128, 512], F32, tag="b1", name="qsum_ps")[0:1, :]
            nc.tensor.matmul(qsum_ps[:], ones_col[:], qacc[:], start=True, stop=True)
            qsum_sb = scpool.tile([1, 512], F32)
            nc.vector.tensor_add(qsum_sb[:], qsum_ps[:], qb_sb[:])

            # ---- qsum row -> columns (128,4) via PE transpose ----
            qt_sb = scpool.tile([128, 4], F32)
            for c in range(4):
                tq = ps1.tile([128, 512], F32, tag="b1")
                nc.tensor.transpose(tq[:, 0:1], qsum_sb[0:1, c * 128:(c + 1) * 128],
                                    ident[0:1, 0:1])
                nc.vector.tensor_copy(qt_sb[:, c:c + 1], tq[:, 0:1])

            # ---- qsumblk[f, h] = qsum[f] * (f//64 == h) ----
            qsblk = scpool.tile([128, 4, 8], F32)
            for c in range(4):
                nc.vector.tensor_scalar_mul(qsblk[:, c, :], bdmask[:, c, :],
                                            qt_sb[:, c:c + 1])

            # ---- u[dm, h] = sum_f wkT[f, dm] qsumblk[f, h] ----
            u_sb = scpool.tile([128, 8, 8], F32)
            for i in range(8):
                u_ps = ps1.tile([128, 512], F32, tag="b1", name="u_ps")[:, 0:8]
                for c in range(4):
                    nc.tensor.matmul(u_ps[:], wkT_sb[:, c, i * 128:(i + 1) * 128],
                                     qsblk[:, c, :], start=(c == 0), stop=(c == 3))
                nc.vector.tensor_copy(u_sb[:, i, :], u_ps[:])

            # ---- scoresT (8, 1024) = u.T @ xkT (xk streamed) ----
            V_sb = bpool.tile([128, 8, 512], F32)
            scoresT_psA = ps2.tile([128, 512], F32, tag="big2",
                                   name="scoresT_psA")[0:8, :]
            scoresT_psB = ps2.tile([128, 512], F32, tag="big2",
                                   name="scoresT_psB")[0:8, :]
            sc_half = (scoresT_psA, scoresT_psB)
            for i in range(8):
                xt = xspool.tile([128, 1024], F32, tag="xk")
                nc.sync.dma_start(xt[:], xkT_d.ap()[i * 128:(i + 1) * 128, :])
                for half in range(2):
                    nc.tensor.matmul(
                        sc_half[half][:],
                        u_sb[:, i, :], xt[:, half * 512:(half + 1) * 512],
                        start=(i == 0), stop=(i == 7))
            # ---- per-block: scores->sbuf copy, chained cummax, m_flat,
            # scols — so attention block 0 can start before later scores
            # chunks are even processed ----
            scoresT_sb = scpool.tile([8, 1024], F32)
            m_all = scpool.tile([8, 1024], F32)
            m_flat = scpool.tile([1, 8192], mybir.dt.bfloat16 if cfg["r_mbcast"] else F32)
            scols = scpool.tile([128, 64], F32)
            for blk in range(NB):
                sl = slice(blk * 128, (blk + 1) * 128)
                hsl = slice((blk % 4) * 128, (blk % 4 + 1) * 128)
                nc.vector.tensor_copy(scoresT_sb[:, sl], sc_half[blk // 4][:, hsl])
                init = -3.0e38 if blk == 0 else m_all[:, blk * 128 - 1:blk * 128]
                nc.vector.tensor_tensor_scan(m_all[:, sl], scoresT_sb[:, sl],
                                             scoresT_sb[:, sl], init,
                                             ALU.max, ALU.max)
                nc.gpsimd.dma_start(
                    out=m_flat[0:1, blk * 1024:(blk + 1) * 1024],
                    in_=m_all[:, sl])
                tp = ps1.tile([128, 512], F32, tag="b1")
                nc.tensor.transpose(tp[:, 0:8], scoresT_sb[:, sl],
                                    ident[0:8, 0:8])
                nc.vector.tensor_copy(scols[:, blk * 8:(blk + 1) * 8], tp[:, 0:8])
            # wo load queued after m_flat so it doesn't delay attention start
            nc.gpsimd.dma_start(
                out=wo_sb[:], in_=wo_d.ap().rearrange("(c p) m -> p c m", p=128))

            # ---- attention: blockwise prefix softmax-average of V ----
            ctx_sb = bpool.tile([128, 8, 512], F32, tag="r1")
            NZ = scpool.tile([1, 65 * NH], F32)
            ctxT_sb = bpool.tile([128, 4, 1024], DTO, name="ctxT_sb")

            def _emit_quarter(q):
                n_t = 0
                for c in range(4):
                    for p8 in (2 * q, 2 * q + 1):
                        tp = ps3.tile([128, 512], F32, tag="b2", name="tp")
                        nc.tensor.transpose(
                            tp[:, 0:128], ctx_sb[:, p8, c * 128:(c + 1) * 128],
                            ident[:])
                        dst = ctxT_sb[:, c, p8 * 128:(p8 + 1) * 128]
                        if n_t % 2 == 0:
                            nc.vector.tensor_copy(dst, tp[:, 0:128])
                        else:
                            nc.scalar.copy(dst, tp[:, 0:128])
                        n_t += 1
                for i in range(8):
                    O_ps = ps3.tile([128, 512], F32, tag="b2",
                                    name="O_ps")[:, 0:256]
                    for c in range(4):
                        nc.tensor.matmul(
                            O_ps[:],
                            wo_sb[:, c, i * 128:(i + 1) * 128],
                            ctxT_sb[:, c, q * 256:(q + 1) * 256],
                            start=(c == 0), stop=(c == 3))
                    oT = opool.tile([128, 256], F32, tag="ot")
                    if i % 2 == 0:
                        nc.vector.tensor_copy(oT[:], O_ps[:])
                    else:
                        nc.scalar.copy(oT[:], O_ps[:])
                    nc.sync.dma_start(
                        outT_d.ap()[i * 128:(i + 1) * 128,
                                    q * 256:(q + 1) * 256], oT[:])
            for blk in range(NB):
                G_h = []
                for half in range(2):
                    G_ps = ps2.tile([128, 512], F32, tag="big2", name="G_ps")
                    nc.tensor.matmul(
                        G_ps[:],
                        ones_row_r[:],
                        m_flat[0:1, blk * 1024 + half * 512:
                               blk * 1024 + (half + 1) * 512],
                        start=True, stop=True)
                    G_h.append(G_ps)
                # just-in-time V projection for this block (after G so the
                # DVE/ACT pipeline for this block is fed first)
                V_ps = ps1.tile([128, 512], F32, tag="b1", name="V_ps")
                for j in range(8):
                    nc.tensor.matmul(
                        V_ps[:],
                        xvT_sb[:, j, blk * 128:(blk + 1) * 128],
                        wv_sb[:, j, :],
                        start=(j == 0), stop=(j == 7),
                    )
                if blk % 2 == 0:
                    nc.vector.tensor_copy(V_sb[:, blk, :], V_ps[:])
                else:
                    nc.scalar.copy(V_sb[:, blk, :], V_ps[:])
                # G[:, h*128+t] = m_r[t] - s[t'] + mask; one batched exp(-G)
                # per block. m_flat is f32r-rounded and the PE's 1.0*m product
                # is exact, so alpha from m_flat matches the F scale exactly.
                G_sb = gfpool.tile([128, 1024], F32, tag="gsb")
                for h in range(NH):
                    idx = blk * 8 + h
                    nc.vector.scalar_tensor_tensor(
                        G_sb[:, h * 128:(h + 1) * 128],
                        G_h[h // 4][:, (h % 4) * 128:(h % 4 + 1) * 128],
                        scols[:, idx:idx + 1], masktri[:],
                        ALU.subtract, ALU.add)
                for half in range(2):
                    nc.scalar.activation(G_sb[:, half * 512:(half + 1) * 512],
                                         G_sb[:, half * 512:(half + 1) * 512],
                                         AF.Exp, scale=-1.0)
                F_all = G_sb
                mb = m_flat
                C_all = ps1.tile([128, 512], F32, tag="b1", name="C_all")
                CZ_all = ps1.tile([128, 512], F32, tag="b1", name="CZ_all")[:, 0:8]
                for h in range(NH):
                    idx = blk * 8 + h
                    base = blk * 1024 + h * 128
                    F_sb = F_all[:, h * 128:(h + 1) * 128]
                    if blk > 0:
                        alpha = gfpool.tile([1, 128], F32, tag="alpha")
                        prev = (blk - 1) * 1024 + h * 128 + 127
                        nc.scalar.activation(alpha[:], m_flat[0:1, base:base + 128],
                                             AF.Exp, scale=-1.0,
                                             bias=m_flat[0:1, prev:prev + 1])
                    # All heads' prefix sums share two PSUM banks (C_all for
                    # the V-parts, CZ_all for the Z columns) so the reciprocal
                    # and the divide batch once per block.
                    nc.tensor.matmul(CZ_all[:, h:h + 1], F_sb, ones_col[:],
                                     start=(h == 0),
                                     stop=(blk == 0 and h == NH - 1))
                    nc.tensor.matmul(C_all[:, h * 64:(h + 1) * 64], F_sb,
                                     V_sb[:, blk, h * 64:(h + 1) * 64],
                                     start=(h == 0),
                                     stop=(blk == 0 and h == NH - 1))
                    # next-block carry row: full-block sums at scale M_new
                    # (not needed after the last block)
                    if blk < NB - 1:
                        NZr_ps = ps1.tile([128, 512], F32, tag="b1",
                                          name="NZr_ps")[0:1, 0:65]
                        Fcol = F_all[:, h * 128 + 127:h * 128 + 128]
                        nc.tensor.matmul(NZr_ps[0:1, 0:64], Fcol,
                                         V_sb[:, blk, h * 64:(h + 1) * 64],
                                         start=True, stop=False)
                        nc.tensor.matmul(NZr_ps[0:1, 64:65], Fcol, ones_col[:],
                                         start=False, stop=(blk == 0))
                    if blk > 0:
                        nc.tensor.matmul(C_all[:, h * 64:(h + 1) * 64], alpha[:],
                                         NZ[0:1, h * 65:h * 65 + 64],
                                         start=False, stop=(h == NH - 1))
                        nc.tensor.matmul(CZ_all[:, h:h + 1], alpha[:],
                                         NZ[0:1, h * 65 + 64:h * 65 + 65],
                                         start=False, stop=(h == NH - 1))
                        if blk < NB - 1:
                            nc.tensor.matmul(NZr_ps[0:1, 0:65], alpha[0:1, 127:128],
                                             NZ[0:1, h * 65:(h + 1) * 65],
                                             start=False, stop=True)
                    if blk < NB - 1:
                        nc.vector.tensor_copy(NZ[0:1, h * 65:(h + 1) * 65], NZr_ps[:])
                zr_all = gfpool.tile([128, 8], F32, tag="zr")
                nc.vector.reciprocal(zr_all[:], CZ_all[:, 0:8])
                for h in range(NH):
                    if h % 2 == 0:
                        nc.scalar.mul(ctx_sb[:, blk, h * 64:(h + 1) * 64],
                                      C_all[:, h * 64:(h + 1) * 64],
                                      zr_all[:, h:h + 1])
                    else:
                        nc.vector.tensor_scalar_mul(
                            ctx_sb[:, blk, h * 64:(h + 1) * 64],
                            C_all[:, h * 64:(h + 1) * 64], zr_all[:, h:h + 1])

                # Out-projection quarters, emitted one block late so their
                # ctx inputs are already finalized (no PE stall) and the PE
                # work overlaps later blocks' DVE/ACT attention work.
                if blk >= 2 and blk % 2 == 0:
                    _emit_quarter(blk // 2 - 1)
            _emit_quarter(3)

    nc.compile()
    return nc


_NC_CACHE = {}


def _get_nc():
    key = tuple(sorted(CFG.items()))
    if key not in _NC_CACHE:
        _NC_CACHE[key] = build(CFG)
    return _NC_CACHE[key]


def _consts():
    p = np.arange(128)
    masktri = np.where(p[:, None] > p[None, :], BIG, 0.0).astype(np.float32)
    ident = np.eye(128, dtype=np.float32)
    bd = np.zeros((128, 4, 8), np.float32)
    for c in range(4):
        for pp in range(128):
            bd[pp, c, 2 * c + pp // 64] = 1.0
    return masktri, ident, bd


def make_in_maps(key_in, query_in, value_in, Wk, bk, Wq, bq, Wv, bv, Wo, bo):
    masktri, ident, bd = _consts()
    maps = []
    for core in range(NC):
        b, hh = core // 2, core % 2
        sl = slice(hh * 512, (hh + 1) * 512)
        maps.append({
            "xqT": np.ascontiguousarray(np.asarray(query_in[b]).T, np.float32),
            "xkT": np.ascontiguousarray(np.asarray(key_in[b]).T, np.float32),
            "xvT": np.ascontiguousarray(np.asarray(value_in[b]).T, np.float32),
            "wq": np.ascontiguousarray(np.asarray(Wq)[:, sl], np.float32),
            "wkT": np.ascontiguousarray(np.asarray(Wk)[:, sl].T, np.float32),
            "wv": np.ascontiguousarray(np.asarray(Wv)[:, sl], np.float32),
            "wo": np.ascontiguousarray(np.asarray(Wo)[sl, :], np.float32),
            "qbS": (S * np.asarray(bq)[sl]).reshape(1, 512).astype(np.float32),
            "masktri": masktri, "ident": ident, "bdmask": bd,
        })
    return maps


def run(inputs, trace=False):
    nc = _get_nc()
    in_maps = make_in_maps(**inputs)
    try:
        res = run_bass_kernel_spmd(nc, in_maps, list(range(NC)), trace=trace)
    except ModuleNotFoundError:
        # Tracing needs antenv.axon_hooks, absent in some containers; retry
        # with tracing suppressed (BASS_TRACE in the env would re-trigger it).
        os.environ["BASS_NEVER_TRACE"] = "1"
        res = run_bass_kernel_spmd(nc, in_maps, list(range(NC)), trace=False)
    Wo = np.asarray(inputs["Wo"], np.float32)
    extra = (np.asarray(inputs["bv"], np.float32) @ Wo
             + np.asarray(inputs["bo"], np.float32)).astype(np.float32)
    out = np.empty((4, S, D), np.float32)
    for b in range(4):
        out[b] = (res.results[2 * b]["outT"].T + res.results[2 * b + 1]["outT"].T
                  + extra)
    return out, res


def kernel(**inputs):
    out, _ = run(inputs, trace=False)
    return out
